# revision 16
# baseline (speedup 1.0000x reference)
"""Distributed multi-head attention kernel for 8 TRN2 NeuronCores.

Problem: x[2,2048,1024] -> qkv proj (w_qkv[3072,1024]) -> 16-head SDPA ->
out proj (w_proj[1024,1024], b_proj[1024]).

Sharding: tensor-parallel over heads. Core c owns heads {2c, 2c+1}:
  - stage 1 (per core): q/k/v for its 2 heads over ALL 4096 tokens,
    transposed score tiles S^T[m,n] per (batch, head), exp on the scalar
    engine (no max-subtraction: scores ~ N(0,1), fp32 exp is safe), PV with
    a trailing ones-column in V so PSUM row DH accumulates the softmax
    denominator, then normalize. Result: aT_h [64 head-dims, 4096 tokens].
  - Four small AllToAlls (one per half-batch, all but the last overlapped
    with compute) reshard from head-parallel to token-parallel: core c
    receives the full 1024 C-dims for its four 128-token shard pieces
    [b*2048 + h*1024 + 128*c, +128).
  - stage 2 (per core): y^T[1024, 512] = w_proj @ a + b_proj for its shard.

Emission order is tuned for the engine queues (strict FIFO per engine):
batch-0's QKV chunks are interleaved INTO batch-0's first attention
n-chunk at m-tile granularity, batch-1's QKV chunks fill the TensorEngine
slack of batch-0's later n-chunks, and resharding/projection overlap
batch-1's attention. The scalar engine (exp) paces the steady state.

All TensorE matmuls run in bf16 (fp32 PSUM accumulation); softmax exp is
computed in fp32 on the scalar engine straight out of PSUM.
"""

import numpy as np
import ml_dtypes

import concourse.bass as bass
import concourse.bacc as bacc
import concourse.tile as tile
import concourse.mybir as mybir
from concourse import bass_utils

BF16 = mybir.dt.bfloat16
F32 = mybir.dt.float32

N_CORES = 8
B = 2
N = 2048
DIM = 1024
H = 16
DH = 64
SCALE = DH ** -0.5
HPC = H // N_CORES          # heads per core = 2
T = B * N                   # 4096 global tokens
TPC = T // N_CORES          # 512 tokens per core in stage 2
SPB = TPC // B              # stage-2 tokens per core per batch = 256
SPP = SPB // 2              # tokens per shard piece = 128
CT = DIM // 128             # 8 contraction tiles
TCH = 512                   # token chunk for stage-1 matmul streaming
NCH = 512                   # n (query) chunk in attention
MT = N // 128               # 16 m-tiles per batch

_cached = None


class _Ctx:
    pass


def _load_chunk(c, tci):
    """DMA one token chunk of x^T into SBUF, split per c-tile so the first
    matmul can start after 1/8 of the transfer."""
    t0 = tci * TCH
    xc = c.xin.tile([128, CT, TCH], BF16, tag="xc", name="xc")
    for a in range(CT):
        c.nc.sync.dma_start(
            xc[:, a, :],
            c.xT_d[128 * a:128 * (a + 1), t0:t0 + TCH])
    return xc


def _k_chunk(c, xc, tci):
    t0 = tci * TCH
    k_ps = c.psA.tile([128, TCH], F32, tag="psA", name="k_ps")
    for a in range(CT):
        c.nc.tensor.matmul(k_ps[:], c.wk_sb[:, a, :], xc[:, a, :],
                           start=(a == 0), stop=(a == CT - 1))
    c.nc.vector.tensor_copy(c.kT[:, t0:t0 + TCH], k_ps[:])


def _qv_chunk(c, xc, tci):
    t0 = tci * TCH
    q_ps = c.psA.tile([128, TCH], F32, tag="psA", name="q_ps")
    for a in range(CT):
        c.nc.tensor.matmul(q_ps[:], c.wq_sb[:, a, :], xc[:, a, :],
                           start=(a == 0), stop=(a == CT - 1))
    c.nc.vector.tensor_copy(c.qT[:, t0:t0 + TCH], q_ps[:])
    for mt in range(TCH // 128):
        gmt = (t0 // 128) + mt
        v_ps = c.psA.tile([128, 128], F32, tag="psA", name="v_ps")
        for a in range(CT):
            c.nc.tensor.matmul(
                v_ps[:], xc[:, a, 128 * mt:128 * (mt + 1)], c.wv_sb[:, a, :],
                start=(a == 0), stop=(a == CT - 1))
        for h in range(HPC):
            c.nc.vector.tensor_copy(
                c.v_aug[h][:, gmt, 0:DH], v_ps[:, DH * h:DH * (h + 1)])


def _attn_qk_exp(c, b, nci, mt):
    nc = c.nc
    n0 = b * N + nci * NCH
    m0 = b * N + 128 * mt
    s_ps = c.psA.tile([128, HPC * NCH], F32, tag="psA", name="s_ps")
    e_t = c.etp.tile([128, HPC * NCH], BF16, tag="et", name="e_t")
    for h in range(HPC):
        nc.tensor.matmul(
            s_ps[:, NCH * h:NCH * (h + 1)],
            c.kT[DH * h:DH * (h + 1), m0:m0 + 128],
            c.qT[DH * h:DH * (h + 1), n0:n0 + NCH],
            start=True, stop=True,
            tile_position=(DH * h, 0))
    nc.scalar.activation(e_t[:], s_ps[:],
                         mybir.ActivationFunctionType.Exp, scale=SCALE)
    return e_t


def _attn_pv(c, mt, gmt, e_t, o_ps):
    nc = c.nc
    for h in range(HPC):
        nc.tensor.matmul(
            o_ps[h][:], c.v_aug[h][:, gmt, :],
            e_t[:, NCH * h:NCH * (h + 1)],
            start=(mt == 0), stop=(mt == MT - 1))


def _attn_normalize(c, b, nci, o_ps):
    """Row DH of o_ps is the softmax denominator. o_ps is copied to SBUF
    first so the PSUM accumulator banks free immediately (the next n-chunk's
    PV never waits on this chain). Then: reciprocal at the aligned
    partition, a small DMA (on the Act DGE ring, so it never queues behind
    bulk loads) moves it to partition 0, gpsimd broadcasts, DVE scales."""
    nc = c.nc
    n0 = b * N + nci * NCH
    osb = c.small.tile([DH + 1, HPC * NCH], F32, tag="osb", name="osb")
    rden = c.small.tile([1, HPC * NCH], F32, tag="rden", name="rden")
    rb = c.small.tile([DH, HPC * NCH], F32, tag="rb", name="rb")
    for h in range(HPC):
        nc.vector.tensor_copy(osb[:, NCH * h:NCH * (h + 1)], o_ps[h][:])
    den = osb[DH:DH + 1, :]
    nc.vector.reciprocal(den, den)
    nc.scalar.dma_start(rden[:], den)
    nc.gpsimd.partition_broadcast(rb[:], rden[:])
    for h in range(HPC):
        nc.vector.tensor_mul(c.aT[h][:, n0:n0 + NCH],
                             osb[0:DH, NCH * h:NCH * (h + 1)],
                             rb[:, NCH * h:NCH * (h + 1)])


def _attn_nchunk(c, b, nci, interleave=None):
    """One attention n-chunk; optional {mt: fn} callbacks emitted after
    given m-tiles to fill TensorEngine slack."""
    o_ps = [c.psB.tile([DH + 1, NCH], F32, tag="psB", name=f"o_ps{h}")
            for h in range(HPC)]
    # software-pipelined: PV of m-tile mt-1 is emitted after QK^T/exp of
    # m-tile mt, so the next QK^T never queues behind an exp-blocked PV.
    prev = None
    for mt in range(MT):
        e_t = _attn_qk_exp(c, b, nci, mt)
        if prev is not None:
            _attn_pv(c, prev[0], b * MT + prev[0], prev[1], o_ps)
        prev = (mt, e_t)
        if interleave and mt in interleave:
            interleave[mt]()
    _attn_pv(c, prev[0], b * MT + prev[0], prev[1], o_ps)
    _attn_normalize(c, b, nci, o_ps)
    # reshard input pieces covered by this n-chunk (4 x 128 tokens)
    half, sub = divmod(nci, 2)
    _a2a_in_writes(c, b, half, range(4 * sub, 4 * sub + 4))
    if sub == 1:
        _reshard_half(c, b, half)


def _a2a_in_writes(c, b, half, js):
    for j in js:
        t0 = b * N + half * (N // 2) + j * SPP
        for h in range(HPC):
            c.nc.scalar.dma_start(c.a2a_in[b][half][j, h, :, :],
                                  c.aT[h][:, t0:t0 + SPP])


def _reshard_half(c, b, half):
    """AllToAll for half-batch (b, half): core j gets C-complete rows for
    its SPP-token piece [b*N + half*N/2 + SPP*j, +SPP)."""
    nc = c.nc
    if c.use_collective:
        nc.gpsimd.collective_compute(
            "AllToAll", mybir.AluOpType.bypass,
            replica_groups=[list(range(N_CORES))],
            ins=[c.a2a_in[b][half].opt()], outs=[c.a2a_out[b][half].opt()])
    else:
        nc.scalar.dma_start(c.a2a_out[b][half][:], c.a2a_in[b][half][:])
    col0 = b * SPB + half * SPP
    nc.scalar.dma_start(
        c.agT[:, :, col0:col0 + SPP],
        c.a2a_out[b][half][:].rearrange("w h d t -> (h d) w t"))


def _proj_cols(c, col0, ncols, ots):
    """Output projection for agT columns [col0, col0+ncols)."""
    nc = c.nc
    for ot in ots:
        y_ps = c.psA.tile([128, SPB], F32, tag="psA", name="y_ps")
        for a in range(CT):
            nc.tensor.matmul(
                y_ps[:, 0:ncols], c.wp_sb[:, a, 128 * ot:128 * (ot + 1)],
                c.agT[:, a, col0:col0 + ncols],
                start=(a == 0), stop=(a == CT - 1))
        y_t = c.outp.tile([128, SPB], BF16, tag="yt", name="y_t")
        nc.vector.tensor_scalar_add(y_t[:, 0:ncols], y_ps[:, 0:ncols],
                                    c.bmat_sb[:, ot:ot + 1])
        nc.scalar.dma_start(
            c.out_d[128 * ot:128 * (ot + 1), col0:col0 + ncols],
            y_t[:, 0:ncols])


def _build(use_collective=True, reps=1):
    """reps>1 unrolls the whole computation N times inside one NEFF —
    used only for timing (differencing out per-execution overhead)."""
    nc = bacc.Bacc("TRN2", target_bir_lowering=False, debug=False,
                   num_devices=N_CORES if use_collective else 1)
    c = _Ctx()
    c.nc = nc
    c.use_collective = use_collective

    c.xT_d = nc.dram_tensor("xT", [DIM, T], BF16, kind="ExternalInput")
    wqT_d = nc.dram_tensor("wqT", [DIM, 128], BF16, kind="ExternalInput")
    wkT_d = nc.dram_tensor("wkT", [DIM, 128], BF16, kind="ExternalInput")
    wvT_d = nc.dram_tensor("wvT", [DIM, 128], BF16, kind="ExternalInput")
    wpT_d = nc.dram_tensor("wpT", [DIM, DIM], BF16, kind="ExternalInput")
    bmat_d = nc.dram_tensor("bmat", [128, CT], F32, kind="ExternalInput")
    c.out_d = nc.dram_tensor("out", [DIM, TPC], BF16, kind="ExternalOutput")

    with tile.TileContext(nc) as tc:
        with (
            tc.tile_pool(name="const", bufs=1) as const,
            tc.tile_pool(name="xin", bufs=7) as xin,
            tc.tile_pool(name="acts", bufs=1) as acts,
            tc.tile_pool(name="et", bufs=6) as etp,
            tc.tile_pool(name="small", bufs=3) as small,
            tc.tile_pool(name="outp", bufs=4) as outp,
            tc.tile_pool(name="psA", bufs=3, space="PSUM") as psA,
            tc.tile_pool(name="psB", bufs=2, space="PSUM") as psB,
            tc.tile_pool(name="dram", bufs=1, space="DRAM") as dram,
        ):
            c.xin, c.etp, c.small, c.outp = xin, etp, small, outp
            c.psA, c.psB = psA, psB

            # ---- constants (k weights first: they gate the critical path) ----
            c.wk_sb = const.tile([128, CT, 128], BF16, name="wk_sb")
            c.wq_sb = const.tile([128, CT, 128], BF16, name="wq_sb")
            c.wv_sb = const.tile([128, CT, 128], BF16, name="wv_sb")
            c.wp_sb = const.tile([128, CT, DIM], BF16, name="wp_sb")
            c.bmat_sb = const.tile([128, CT], F32, name="bmat_sb")
            nc.sync.dma_start(c.wk_sb[:],
                              wkT_d.ap().rearrange("(a p) m -> p a m", p=128))
            nc.sync.dma_start(c.wq_sb[:],
                              wqT_d.ap().rearrange("(a p) m -> p a m", p=128))
            nc.sync.dma_start(c.wv_sb[:],
                              wvT_d.ap().rearrange("(a p) m -> p a m", p=128))
            nc.sync.dma_start(c.bmat_sb[:], bmat_d[:])
            nc.scalar.dma_start(
                c.wp_sb[:], wpT_d.ap().rearrange("(a p) m -> p a m", p=128))

            # persistent activations
            c.qT = acts.tile([128, T], BF16, name="qT")
            c.kT = acts.tile([128, T], BF16, name="kT")
            c.v_aug = [acts.tile([128, T // 128, DH + 1], BF16,
                                 name=f"v_aug{h}") for h in range(HPC)]
            c.aT = [acts.tile([DH, T], BF16, name=f"aT{h}")
                    for h in range(HPC)]
            c.agT = acts.tile([128, CT, TPC], BF16, name="agT")

            for h in range(HPC):
                nc.vector.memset(c.v_aug[h][:, :, DH:DH + 1], 1.0)

            # warmup: a few dummy matmuls raise the PE HAM clock gate to
            # 8/8 and a dummy exp preloads the ACT table set, all during
            # the initial x DMA wait.
            warm = acts.tile([128, 512], BF16, name="warm")
            nc.vector.memset(warm[:], 0.0)
            wm_ps = psA.tile([128, 512], F32, tag="psA", name="wm_ps")
            for _w in range(14):
                nc.tensor.matmul(wm_ps[:], warm[:, 0:128], warm[:],
                                 start=(_w == 0), stop=(_w == 13))
            we_t = etp.tile([128, 512], BF16, tag="et", name="we_t")
            nc.scalar.activation(we_t[:], wm_ps[:],
                                 mybir.ActivationFunctionType.Exp)

            c.a2a_in = [[dram.tile([N_CORES, HPC, DH, SPP], BF16,
                                   name=f"a2a_in{b}{hf}") for hf in range(2)]
                        for b in range(B)]
            c.a2a_out = [[dram.tile([N_CORES, HPC, DH, SPP], BF16,
                                    name=f"a2a_out{b}{hf}") for hf in range(2)]
                         for b in range(B)]

            for _rep in range(reps):
                # batch 0: chunk 0's k/q/v, then attention nc0 with the
                # remaining b0 chunks interleaved at m-tile granularity
                # (QK^T of m-tile mt needs k of chunk mt//4).
                xcs = {0: _load_chunk(c, 0)}
                _k_chunk(c, xcs[0], 0)
                _qv_chunk(c, xcs[0], 0)

                def _mk(tci, drop):
                    def f():
                        xcs[tci] = _load_chunk(c, tci)
                        _k_chunk(c, xcs[tci], tci)
                        _qv_chunk(c, xcs[tci], tci)
                        if drop in xcs:
                            xcs.pop(drop)
                    return f
                _attn_nchunk(c, 0, 0,
                             interleave={0: _mk(1, -1), 4: _mk(2, 0),
                                         8: _mk(3, 1)})
                _attn_nchunk(c, 0, 1, interleave={1: _mk(4, 2), 9: _mk(5, 3)})
                _attn_nchunk(c, 0, 2, interleave={1: _mk(6, 4)})
                _attn_nchunk(c, 0, 3, interleave={1: _mk(7, 5)})

                # batch-1 attention; batch-0 projection interleaved
                _attn_nchunk(c, 1, 0, interleave={
                    4: lambda: _proj_cols(c, 0, SPB, range(0, 2)),
                    8: lambda: _proj_cols(c, 0, SPB, range(2, 4))})
                _attn_nchunk(c, 1, 1, interleave={
                    4: lambda: _proj_cols(c, 0, SPB, range(4, 6)),
                    8: lambda: _proj_cols(c, 0, SPB, range(6, CT))})
                _attn_nchunk(c, 1, 2)
                _attn_nchunk(c, 1, 3)
                # b1-half0 projection runs on PE while the last AllToAll
                # (b1-half1) is in flight, keeping the HAM clock warm
                _proj_cols(c, SPB, SPP, range(CT))
                _proj_cols(c, SPB + SPP, SPP, range(CT))

    nc.compile()
    return nc


def _prep_inputs(x, w_qkv, w_proj, b_proj):
    xf = np.ascontiguousarray(x.reshape(T, DIM).T).astype(ml_dtypes.bfloat16)
    wpT = np.ascontiguousarray(w_proj.T).astype(ml_dtypes.bfloat16)
    bmat = np.ascontiguousarray(b_proj.reshape(CT, 128).T).astype(np.float32)
    in_maps = []
    for c in range(N_CORES):
        r0 = 128 * c
        wqT = np.ascontiguousarray(
            w_qkv[r0:r0 + 128, :].T).astype(ml_dtypes.bfloat16)
        wkT = np.ascontiguousarray(
            w_qkv[DIM + r0:DIM + r0 + 128, :].T).astype(ml_dtypes.bfloat16)
        wvT = np.ascontiguousarray(
            w_qkv[2 * DIM + r0:2 * DIM + r0 + 128, :].T).astype(ml_dtypes.bfloat16)
        in_maps.append({
            "xT": xf, "wqT": wqT, "wkT": wkT, "wvT": wvT,
            "wpT": wpT, "bmat": bmat,
        })
    return in_maps


def _assemble(results):
    out = np.empty((T, DIM), dtype=np.float32)
    for c in range(N_CORES):
        yT = np.asarray(results[c]["out"], dtype=np.float32)  # [DIM, TPC]
        for b in range(B):
            for hf in range(2):
                t0 = b * N + hf * (N // 2) + c * SPP
                col0 = b * SPB + hf * SPP
                out[t0:t0 + SPP, :] = yT[:, col0:col0 + SPP].T
    return out.reshape(B, N, DIM)


def kernel(x, w_qkv, w_proj, b_proj):
    global _cached
    x = np.asarray(x, dtype=np.float32)
    w_qkv = np.asarray(w_qkv, dtype=np.float32)
    w_proj = np.asarray(w_proj, dtype=np.float32)
    b_proj = np.asarray(b_proj, dtype=np.float32)

    if _cached is None:
        _cached = _build()
    nc = _cached

    in_maps = _prep_inputs(x, w_qkv, w_proj, b_proj)
    res = bass_utils.run_bass_kernel_spmd(
        nc, in_maps, core_ids=list(range(N_CORES)))
    return _assemble(res.results)


if __name__ == "__main__":
    import jax
    with jax.default_device(jax.devices("cpu")[0]):
        import reference
        inputs = {k: np.asarray(v) for k, v in reference.setup_inputs().items()}
        expected = np.asarray(reference.reference(**inputs))
    actual = kernel(**inputs)
    err = np.linalg.norm(actual - expected) / np.linalg.norm(expected)
    print("Relative error:", err)


# revision 17
# speedup vs baseline: 1.1071x; 1.1071x over previous
"""Distributed multi-head attention kernel for 8 TRN2 NeuronCores.

Problem: x[2,2048,1024] -> qkv proj (w_qkv[3072,1024]) -> 16-head SDPA ->
out proj (w_proj[1024,1024], b_proj[1024]).

Sharding: tensor-parallel over heads. Core c owns heads {2c, 2c+1}:
  - stage 1 (per core): q/k/v for its 2 heads over ALL 4096 tokens,
    transposed score tiles S^T[m,n] per (batch, head), exp on the scalar
    engine (no max-subtraction: scores ~ N(0,1), fp32 exp is safe), PV with
    a trailing ones-column in V so PSUM row DH accumulates the softmax
    denominator, then normalize. Result: aT_h [64 head-dims, 4096 tokens].
  - Four small AllToAlls (one per half-batch, all but the last overlapped
    with compute) reshard from head-parallel to token-parallel: core c
    receives the full 1024 C-dims for its four 128-token shard pieces
    [b*2048 + h*1024 + 128*c, +128).
  - stage 2 (per core): y^T[1024, 512] = w_proj @ a + b_proj for its shard.

Emission order is tuned for the engine queues (strict FIFO per engine):
batch-0's QKV chunks are interleaved INTO batch-0's first attention
n-chunk at m-tile granularity, batch-1's QKV chunks fill the TensorEngine
slack of batch-0's later n-chunks, and resharding/projection overlap
batch-1's attention. The scalar engine (exp) paces the steady state.

All TensorE matmuls run in bf16 (fp32 PSUM accumulation); softmax exp is
computed in fp32 on the scalar engine straight out of PSUM.
"""

import numpy as np
import ml_dtypes

import concourse.bass as bass
import concourse.bacc as bacc
import concourse.tile as tile
import concourse.mybir as mybir
from concourse import bass_utils

BF16 = mybir.dt.bfloat16
F32 = mybir.dt.float32

N_CORES = 8
B = 2
N = 2048
DIM = 1024
H = 16
DH = 64
SCALE = DH ** -0.5
HPC = H // N_CORES          # heads per core = 2
T = B * N                   # 4096 global tokens
TPC = T // N_CORES          # 512 tokens per core in stage 2
SPB = TPC // B              # stage-2 tokens per core per batch = 256
SPP = SPB // 2              # tokens per shard piece = 128
CT = DIM // 128             # 8 contraction tiles
TCH = 512                   # token chunk for stage-1 matmul streaming
NCH = 512                   # n (query) chunk in attention
MT = N // 128               # 16 m-tiles per batch

_cached = None


class _Ctx:
    pass


def _load_chunk(c, tci):
    """DMA one token chunk of x^T into SBUF, split per c-tile so the first
    matmul can start after 1/8 of the transfer."""
    t0 = tci * TCH
    xc = c.xin.tile([128, CT, TCH], BF16, tag="xc", name="xc")
    for a in range(CT):
        c.nc.sync.dma_start(
            xc[:, a, :],
            c.xT_d[128 * a:128 * (a + 1), t0:t0 + TCH])
    return xc


def _k_chunk(c, xc, tci):
    t0 = tci * TCH
    k_ps = c.psA.tile([128, TCH], F32, tag="psA", name="k_ps")
    for a in range(CT):
        c.nc.tensor.matmul(k_ps[:], c.wk_sb[:, a, :], xc[:, a, :],
                           start=(a == 0), stop=(a == CT - 1))
    c.nc.vector.tensor_copy(c.kT[:, t0:t0 + TCH], k_ps[:])


def _qv_chunk(c, xc, tci):
    t0 = tci * TCH
    q_ps = c.psA.tile([128, TCH], F32, tag="psA", name="q_ps")
    for a in range(CT):
        c.nc.tensor.matmul(q_ps[:], c.wq_sb[:, a, :], xc[:, a, :],
                           start=(a == 0), stop=(a == CT - 1))
    c.nc.vector.tensor_copy(c.qT[:, t0:t0 + TCH], q_ps[:])
    for mt in range(TCH // 128):
        gmt = (t0 // 128) + mt
        v_ps = c.psA.tile([128, 128], F32, tag="psA", name="v_ps")
        for a in range(CT):
            c.nc.tensor.matmul(
                v_ps[:], xc[:, a, 128 * mt:128 * (mt + 1)], c.wv_sb[:, a, :],
                start=(a == 0), stop=(a == CT - 1))
        for h in range(HPC):
            c.nc.vector.tensor_copy(
                c.v_aug[h][:, gmt, 0:DH], v_ps[:, DH * h:DH * (h + 1)])


def _attn_qk_exp(c, b, nci, mt):
    nc = c.nc
    n0 = b * N + nci * NCH
    m0 = b * N + 128 * mt
    s_ps = c.psA.tile([128, HPC * NCH], F32, tag="psA", name="s_ps")
    e_t = c.etp.tile([128, HPC * NCH], BF16, tag="et", name="e_t")
    for h in range(HPC):
        nc.tensor.matmul(
            s_ps[:, NCH * h:NCH * (h + 1)],
            c.kT[DH * h:DH * (h + 1), m0:m0 + 128],
            c.qT[DH * h:DH * (h + 1), n0:n0 + NCH],
            start=True, stop=True,
            tile_position=(DH * h, 0))
    nc.scalar.activation(e_t[:], s_ps[:],
                         mybir.ActivationFunctionType.Exp, scale=SCALE)
    return e_t


def _attn_pv(c, mt, gmt, e_t, o_ps):
    nc = c.nc
    for h in range(HPC):
        nc.tensor.matmul(
            o_ps[h][:], c.v_aug[h][:, gmt, :],
            e_t[:, NCH * h:NCH * (h + 1)],
            start=(mt == 0), stop=(mt == MT - 1))


def _attn_normalize(c, b, nci, o_ps):
    """Row DH of o_ps is the softmax denominator. Reciprocal at the aligned
    partition, a small DMA (on the Act DGE ring, so it never queues behind
    bulk loads) moves it to partition 0, gpsimd broadcasts, DVE scales."""
    nc = c.nc
    n0 = b * N + nci * NCH
    den = c.small.tile([DH + 1, HPC * NCH], F32, tag="den", name="den")
    rden = c.small.tile([1, HPC * NCH], F32, tag="rden", name="rden")
    rb = c.small.tile([DH, HPC * NCH], F32, tag="rb", name="rb")
    for h in range(HPC):
        nc.vector.reciprocal(den[DH:DH + 1, NCH * h:NCH * (h + 1)],
                             o_ps[h][DH:DH + 1, :])
    nc.scalar.dma_start(rden[:], den[DH:DH + 1, :])
    nc.gpsimd.partition_broadcast(rb[:], rden[:])
    for h in range(HPC):
        nc.vector.tensor_mul(c.aT[h][:, n0:n0 + NCH],
                             o_ps[h][0:DH, :],
                             rb[:, NCH * h:NCH * (h + 1)])


def _attn_nchunk(c, b, nci, interleave=None):
    """One attention n-chunk; optional {mt: fn} callbacks emitted after
    given m-tiles to fill TensorEngine slack."""
    o_ps = [c.psB.tile([DH + 1, NCH], F32, tag="psB", name=f"o_ps{h}")
            for h in range(HPC)]
    # software-pipelined: PV of m-tile mt-1 is emitted after QK^T/exp of
    # m-tile mt, so the next QK^T never queues behind an exp-blocked PV.
    prev = None
    for mt in range(MT):
        e_t = _attn_qk_exp(c, b, nci, mt)
        if prev is not None:
            _attn_pv(c, prev[0], b * MT + prev[0], prev[1], o_ps)
        prev = (mt, e_t)
        if interleave and mt in interleave:
            interleave[mt]()
    _attn_pv(c, prev[0], b * MT + prev[0], prev[1], o_ps)
    _attn_normalize(c, b, nci, o_ps)
    # reshard input pieces covered by this n-chunk (4 x 128 tokens)
    half, sub = divmod(nci, 2)
    _a2a_in_writes(c, b, half, range(4 * sub, 4 * sub + 4))
    if sub == 1:
        _reshard_half(c, b, half)


def _a2a_in_writes(c, b, half, js):
    for j in js:
        t0 = b * N + half * (N // 2) + j * SPP
        for h in range(HPC):
            c.nc.scalar.dma_start(c.a2a_in[b][half][j, h, :, :],
                                  c.aT[h][:, t0:t0 + SPP])


def _reshard_half(c, b, half):
    """AllToAll for half-batch (b, half): core j gets C-complete rows for
    its SPP-token piece [b*N + half*N/2 + SPP*j, +SPP)."""
    nc = c.nc
    if c.use_collective:
        nc.gpsimd.collective_compute(
            "AllToAll", mybir.AluOpType.bypass,
            replica_groups=[list(range(N_CORES))],
            ins=[c.a2a_in[b][half].opt()], outs=[c.a2a_out[b][half].opt()])
    else:
        nc.scalar.dma_start(c.a2a_out[b][half][:], c.a2a_in[b][half][:])
    col0 = b * SPB + half * SPP
    nc.scalar.dma_start(
        c.agT[:, :, col0:col0 + SPP],
        c.a2a_out[b][half][:].rearrange("w h d t -> (h d) w t"))


def _proj_cols(c, col0, ncols, ots):
    """Output projection for agT columns [col0, col0+ncols)."""
    nc = c.nc
    for ot in ots:
        y_ps = c.psA.tile([128, SPB], F32, tag="psA", name="y_ps")
        for a in range(CT):
            nc.tensor.matmul(
                y_ps[:, 0:ncols], c.wp_sb[:, a, 128 * ot:128 * (ot + 1)],
                c.agT[:, a, col0:col0 + ncols],
                start=(a == 0), stop=(a == CT - 1))
        y_t = c.outp.tile([128, SPB], BF16, tag="yt", name="y_t")
        nc.vector.tensor_scalar_add(y_t[:, 0:ncols], y_ps[:, 0:ncols],
                                    c.bmat_sb[:, ot:ot + 1])
        nc.scalar.dma_start(
            c.out_d[128 * ot:128 * (ot + 1), col0:col0 + ncols],
            y_t[:, 0:ncols])


def _build(use_collective=True, reps=1):
    """reps>1 unrolls the whole computation N times inside one NEFF —
    used only for timing (differencing out per-execution overhead)."""
    nc = bacc.Bacc("TRN2", target_bir_lowering=False, debug=False,
                   num_devices=N_CORES if use_collective else 1)
    c = _Ctx()
    c.nc = nc
    c.use_collective = use_collective

    c.xT_d = nc.dram_tensor("xT", [DIM, T], BF16, kind="ExternalInput")
    wqT_d = nc.dram_tensor("wqT", [DIM, 128], BF16, kind="ExternalInput")
    wkT_d = nc.dram_tensor("wkT", [DIM, 128], BF16, kind="ExternalInput")
    wvT_d = nc.dram_tensor("wvT", [DIM, 128], BF16, kind="ExternalInput")
    wpT_d = nc.dram_tensor("wpT", [DIM, DIM], BF16, kind="ExternalInput")
    bmat_d = nc.dram_tensor("bmat", [128, CT], F32, kind="ExternalInput")
    c.out_d = nc.dram_tensor("out", [DIM, TPC], BF16, kind="ExternalOutput")

    with tile.TileContext(nc) as tc:
        with (
            tc.tile_pool(name="const", bufs=1) as const,
            tc.tile_pool(name="xin", bufs=7) as xin,
            tc.tile_pool(name="acts", bufs=1) as acts,
            tc.tile_pool(name="et", bufs=6) as etp,
            tc.tile_pool(name="small", bufs=3) as small,
            tc.tile_pool(name="outp", bufs=4) as outp,
            tc.tile_pool(name="psA", bufs=3, space="PSUM") as psA,
            tc.tile_pool(name="psB", bufs=2, space="PSUM") as psB,
            tc.tile_pool(name="dram", bufs=1, space="DRAM") as dram,
        ):
            c.xin, c.etp, c.small, c.outp = xin, etp, small, outp
            c.psA, c.psB = psA, psB

            # ---- constants (k weights first: they gate the critical path) ----
            c.wk_sb = const.tile([128, CT, 128], BF16, name="wk_sb")
            c.wq_sb = const.tile([128, CT, 128], BF16, name="wq_sb")
            c.wv_sb = const.tile([128, CT, 128], BF16, name="wv_sb")
            c.wp_sb = const.tile([128, CT, DIM], BF16, name="wp_sb")
            c.bmat_sb = const.tile([128, CT], F32, name="bmat_sb")
            nc.sync.dma_start(c.wk_sb[:],
                              wkT_d.ap().rearrange("(a p) m -> p a m", p=128))
            nc.sync.dma_start(c.wq_sb[:],
                              wqT_d.ap().rearrange("(a p) m -> p a m", p=128))
            nc.sync.dma_start(c.wv_sb[:],
                              wvT_d.ap().rearrange("(a p) m -> p a m", p=128))
            nc.sync.dma_start(c.bmat_sb[:], bmat_d[:])
            nc.scalar.dma_start(
                c.wp_sb[:], wpT_d.ap().rearrange("(a p) m -> p a m", p=128))

            # persistent activations
            c.qT = acts.tile([128, T], BF16, name="qT")
            c.kT = acts.tile([128, T], BF16, name="kT")
            c.v_aug = [acts.tile([128, T // 128, DH + 1], BF16,
                                 name=f"v_aug{h}") for h in range(HPC)]
            c.aT = [acts.tile([DH, T], BF16, name=f"aT{h}")
                    for h in range(HPC)]
            c.agT = acts.tile([128, CT, TPC], BF16, name="agT")

            for h in range(HPC):
                nc.vector.memset(c.v_aug[h][:, :, DH:DH + 1], 1.0)

            # warmup: a few dummy matmuls raise the PE HAM clock gate to
            # 8/8 and a dummy exp preloads the ACT table set, all during
            # the initial x DMA wait.
            warm = acts.tile([128, 512], BF16, name="warm")
            nc.vector.memset(warm[:], 0.0)
            wm_ps = psA.tile([128, 512], F32, tag="psA", name="wm_ps")
            for _w in range(14):
                nc.tensor.matmul(wm_ps[:], warm[:, 0:128], warm[:],
                                 start=(_w == 0), stop=(_w == 13))
            we_t = etp.tile([128, 512], BF16, tag="et", name="we_t")
            nc.scalar.activation(we_t[:], wm_ps[:],
                                 mybir.ActivationFunctionType.Exp)

            c.a2a_in = [[dram.tile([N_CORES, HPC, DH, SPP], BF16,
                                   name=f"a2a_in{b}{hf}") for hf in range(2)]
                        for b in range(B)]
            c.a2a_out = [[dram.tile([N_CORES, HPC, DH, SPP], BF16,
                                    name=f"a2a_out{b}{hf}") for hf in range(2)]
                         for b in range(B)]

            for _rep in range(reps):
                # batch 0: chunk 0's k/q/v, then attention nc0 with the
                # remaining b0 chunks interleaved at m-tile granularity
                # (QK^T of m-tile mt needs k of chunk mt//4).
                xcs = {0: _load_chunk(c, 0)}
                _k_chunk(c, xcs[0], 0)
                _qv_chunk(c, xcs[0], 0)

                def _mk(tci, drop):
                    def f():
                        xcs[tci] = _load_chunk(c, tci)
                        _k_chunk(c, xcs[tci], tci)
                        _qv_chunk(c, xcs[tci], tci)
                        if drop in xcs:
                            xcs.pop(drop)
                    return f
                _attn_nchunk(c, 0, 0,
                             interleave={0: _mk(1, -1), 4: _mk(2, 0),
                                         8: _mk(3, 1)})
                _attn_nchunk(c, 0, 1, interleave={1: _mk(4, 2), 9: _mk(5, 3)})
                _attn_nchunk(c, 0, 2, interleave={1: _mk(6, 4)})
                _attn_nchunk(c, 0, 3, interleave={1: _mk(7, 5)})

                # batch-1 attention; batch-0 projection interleaved
                _attn_nchunk(c, 1, 0, interleave={
                    4: lambda: _proj_cols(c, 0, SPB, range(0, 2)),
                    8: lambda: _proj_cols(c, 0, SPB, range(2, 4))})
                _attn_nchunk(c, 1, 1, interleave={
                    4: lambda: _proj_cols(c, 0, SPB, range(4, 6)),
                    8: lambda: _proj_cols(c, 0, SPB, range(6, CT))})
                _attn_nchunk(c, 1, 2)
                _attn_nchunk(c, 1, 3)
                # b1-half0 projection runs on PE while the last AllToAll
                # (b1-half1) is in flight, keeping the HAM clock warm
                _proj_cols(c, SPB, SPP, range(CT))
                _proj_cols(c, SPB + SPP, SPP, range(CT))

    nc.compile()
    return nc


def _prep_inputs(x, w_qkv, w_proj, b_proj):
    xf = np.ascontiguousarray(x.reshape(T, DIM).T).astype(ml_dtypes.bfloat16)
    wpT = np.ascontiguousarray(w_proj.T).astype(ml_dtypes.bfloat16)
    bmat = np.ascontiguousarray(b_proj.reshape(CT, 128).T).astype(np.float32)
    in_maps = []
    for c in range(N_CORES):
        r0 = 128 * c
        wqT = np.ascontiguousarray(
            w_qkv[r0:r0 + 128, :].T).astype(ml_dtypes.bfloat16)
        wkT = np.ascontiguousarray(
            w_qkv[DIM + r0:DIM + r0 + 128, :].T).astype(ml_dtypes.bfloat16)
        wvT = np.ascontiguousarray(
            w_qkv[2 * DIM + r0:2 * DIM + r0 + 128, :].T).astype(ml_dtypes.bfloat16)
        in_maps.append({
            "xT": xf, "wqT": wqT, "wkT": wkT, "wvT": wvT,
            "wpT": wpT, "bmat": bmat,
        })
    return in_maps


def _assemble(results):
    out = np.empty((T, DIM), dtype=np.float32)
    for c in range(N_CORES):
        yT = np.asarray(results[c]["out"], dtype=np.float32)  # [DIM, TPC]
        for b in range(B):
            for hf in range(2):
                t0 = b * N + hf * (N // 2) + c * SPP
                col0 = b * SPB + hf * SPP
                out[t0:t0 + SPP, :] = yT[:, col0:col0 + SPP].T
    return out.reshape(B, N, DIM)


def kernel(x, w_qkv, w_proj, b_proj):
    global _cached
    x = np.asarray(x, dtype=np.float32)
    w_qkv = np.asarray(w_qkv, dtype=np.float32)
    w_proj = np.asarray(w_proj, dtype=np.float32)
    b_proj = np.asarray(b_proj, dtype=np.float32)

    if _cached is None:
        _cached = _build()
    nc = _cached

    in_maps = _prep_inputs(x, w_qkv, w_proj, b_proj)
    res = bass_utils.run_bass_kernel_spmd(
        nc, in_maps, core_ids=list(range(N_CORES)))
    return _assemble(res.results)


if __name__ == "__main__":
    import jax
    with jax.default_device(jax.devices("cpu")[0]):
        import reference
        inputs = {k: np.asarray(v) for k, v in reference.setup_inputs().items()}
        expected = np.asarray(reference.reference(**inputs))
    actual = kernel(**inputs)
    err = np.linalg.norm(actual - expected) / np.linalg.norm(expected)
    print("Relative error:", err)
